# revision 6
# baseline (speedup 1.0000x reference)
"""CenterLoss Trainium2 kernel v2 (raw bass, data-parallel over 8 cores).

loss = sum(clip(distmat * onehot(labels), 1e-12, 1e12)) / B with
distmat[i,c] = ||x_i - centers_c||^2. Only (i, labels_i) entries survive the
mask; the B*(C-1) masked zeros contribute exactly 1e-12 each (added on host).

  sum_i d_i = sum_i ||x_i||^2 + sum_c n_c ||c_c||^2 - 2 sum_c <s_c, c_c>

with s = onehot^T @ x, n_c per-core class counts. Host sorts samples by label
so each core's 1024 samples span a <=128-class window; x ships fp8.

v2 vs the Tile baseline (13813ns -> ~11.5us):
* Raw bass, manual semaphores. No TileContext: the Tile epilogue (double
  barrier round after the output DMA, ~720ns) disappears, and the SWDGE
  prepare/trigger path becomes usable.
* Output via kv_writeback prepared early (descriptor gen off the critical
  path) + trigger_dma after the last drain: post-accum tail is
  ~40ns dispatch + 4ns transfer + 917ns DMA-completion sem, vs the HWDGE
  dma_start chain (650 SEQ + 625 gen + 650 DGE delay + copy + 917).
* Column-slice-major streaming: x arrives as 7 slabs (512,512,256,256,256,
  128,128 cols) x all 8 tiles, host-packed contiguous so every DMA moves
  >=1KB/descriptor. Each slab completes a narrow PSUM region whose drain
  (DVE stt) runs while later slabs still stream -- the drain pipeline never
  piles up at the end (the last drain is only 128 wide).
* Center-norm term folded into the class matmuls: a DoubleRow diag pair
  (diag(n>>2), diag(n&3)) x (cv, cv/4) injects (n_c/4)*(-2c) into ps, so
  the single drain ps.cv yields -2<s,c> + n||c||^2. All values fp8e4m3-exact
  (n <= 67; cv/4 is a power-of-two scale computed by the otherwise-idle Act
  engine in 512-col chunks chasing the cv DMA).
* sum||x||^2 on the PE as before: fp8 DoubleRow Gram matmuls of every
  128-col block against itself, all accumulated into one [128,128] PSUM
  whose masked diag drain is the final 128-wide stt.

Per-core output: [128, 8] f32 raw partial columns (one per drain).
Host combine (f64): sum + B*(C-1)*1e-12, divide by B.
"""

import ml_dtypes
import numpy as np

import concourse.bacc as bacc
from concourse import mybir
from concourse.bass_utils import run_bass_kernel_spmd

N_CORES = 8
B = 8192
D = 2048
C = 751
BS = B // N_CORES
P = 128
NT = BS // P          # 8 sample tiles per core
KDR = NT // 2         # 4 DoubleRow pairs
W = 128               # class window width
OUTW = 8
FP8 = mybir.dt.float8e4
F32 = mybir.dt.float32
BF16 = mybir.dt.bfloat16
NP_FP8 = ml_dtypes.float8_e4m3
CLIP_LO = 1e-12

# column slabs: starts/widths; slab s covers x cols [SS[s], SS[s]+SW[s])
SW = [512, 512, 256, 256, 256, 128, 128]
SS = [0, 512, 1024, 1280, 1536, 1792, 1920]
NS = len(SW)
# gram 128-col blocks per slab
GB = [w // P for w in SW]  # [4,4,2,2,2,1,1]
NWARM = 70

_NC = None


def build_nc():
    nc = bacc.Bacc("TRN2", target_bir_lowering=False)

    aux_d = nc.dram_tensor("aux", [P, 10], F32, kind="ExternalInput")
    cv_d = nc.dram_tensor("cvq", [P, D], FP8, kind="ExternalInput")
    xs_d = [
        nc.dram_tensor(f"xs{s}", [P, NT, SW[s]], FP8, kind="ExternalInput")
        for s in range(NS)
    ]
    out_d = nc.dram_tensor("part", [1, P, 1, OUTW], F32, kind="ExternalOutput")

    # SBUF
    auxt = nc.alloc_sbuf_tensor("auxt", [P, 10], F32)
    # cv tile-pair for the diag DoubleRow: tile0 = fp8(-2c) (DMA), tile1 =
    # tile0 * 0.25 written by the otherwise-idle Act engine (exact
    # power-of-two scale). The diag pair (diag(n>>2), diag(n&3)) x
    # (cv, cv/4) injects (n/4)*(-2c) exactly.
    cvt2 = nc.alloc_sbuf_tensor("cvt2", [P, 2, D], FP8)
    actscr = nc.alloc_sbuf_tensor("actscr", [P, W], F32)
    xst = [nc.alloc_sbuf_tensor(f"xst{s}", [P, NT, SW[s]], FP8) for s in range(NS)]
    oh = nc.alloc_sbuf_tensor("oh", [P, NT, W], FP8)
    iota_i = nc.alloc_sbuf_tensor("iota_i", [P, W], mybir.dt.int32)
    iota_f = nc.alloc_sbuf_tensor("iota_f", [P, W], F32)
    # identraw[p, q] = q - p via iota; ident = (identraw == 0). No
    # scalar-pointer operand (a same-engine back-to-back scalar-ptr read
    # races with its producing write on hardware dispatch).
    ident_i = nc.alloc_sbuf_tensor("ident_i", [P, W], mybir.dt.int32)
    ident_f = nc.alloc_sbuf_tensor("ident_f", [P, W], F32)
    identf = nc.alloc_sbuf_tensor("identf", [P, W], F32)
    ident8t = nc.alloc_sbuf_tensor("ident8t", [P, W], FP8)
    ident8 = ident8t[:]
    # diag pair, tile-adjacent for DoubleRow lhsT
    dg2 = nc.alloc_sbuf_tensor("dg2", [P, 2, W], FP8)
    junk = nc.alloc_sbuf_tensor("junk", [P, 2, W], FP8)
    outsb = nc.alloc_sbuf_tensor("outsb", [P, 1, 1, OUTW], F32)
    idxt = nc.alloc_sbuf_tensor("idxt", [P, 1], mybir.dt.int32)
    csc = nc.alloc_sbuf_tensor("csc", [P, 512], BF16)

    # PSUM: one accumulation group per bank. 8 banks: A0, A1 (ps_a),
    # S2..S6 (one each), gram sharing the junk-warmup bank (junk's group
    # closes before the gram group starts).
    ps_a = nc.alloc_psum_tensor("ps_a", [P, 1024], F32)   # A0 [0:512], A1 [512:1024]
    ps_s = [nc.alloc_psum_tensor(f"ps_s{s}", [P, SW[s]], F32) for s in range(2, NS)]
    ps_gj = nc.alloc_psum_tensor("ps_gj", [P, W], F32)

    def region(s):
        """PSUM AP of slab s's class accumulation."""
        if s < 2:
            return ps_a[:, 512 * s : 512 * (s + 1)]
        return ps_s[s - 2][:]

    ps_g = ps_gj[:]
    # junk warmup borrows S5's bank: its group closes ~4.4us before the S5
    # class group opens (widest reopen margin of any bank)
    ps_j = ps_s[3][:]

    # one completion sem PER DMA: the functional executor may complete DMAs
    # out of order, so a single cumulative sem can release a consumer before
    # ITS data has landed (reads uninitialized SBUF -> NaN)
    s_d = {k: nc.alloc_semaphore(f"s_d_{k}")
           for k in ["x0", "aux", "cv"] + [f"x{s}" for s in range(1, NS)]}
    s_junk = nc.alloc_semaphore("s_junk")
    s_iota = nc.alloc_semaphore("s_iota")
    s_oh = nc.alloc_semaphore("s_oh")      # one-hots + diag tiles ready
    s_cv4 = nc.alloc_semaphore("s_cv4")    # cv/4 tile ready
    s_pe = nc.alloc_semaphore("s_pe")      # region stops, cumulative
    s_dve = nc.alloc_semaphore("s_dve")    # DVE's last drain done
    s_prep = nc.alloc_semaphore("s_prep")  # writeback descriptors ready
    s_dma = nc.alloc_semaphore("s_dma")    # writeback done

    # ---------------- Pool: setup + prepped output writeback ----------------
    nc.gpsimd.memset(junk[:], 0.0).then_inc(s_junk, 1)
    nc.gpsimd.iota(iota_i[:], pattern=[[1, W]], base=0, channel_multiplier=0).then_inc(
        s_iota, 1
    )
    nc.gpsimd.iota(ident_i[:], pattern=[[1, W]], base=0, channel_multiplier=-1).then_inc(
        s_iota, 1
    )
    nc.gpsimd.memset(idxt[:], 0)
    nc.gpsimd.kv_writeback(
        out_ap=out_d[:], in_ap=outsb[:], ctx_idxs_ap=idxt[:],
        prepare_only=True, sem=s_dma,
    ).then_inc(s_prep, 1)

    # ---------------- SP: input stream ----------------
    # xs0 first: a small first DMA would leave the wire idle while the
    # second DMA's descriptor-gen (+650ns DGE delay) pipeline fills.
    def dma(dst, src, key):
        nc.sync.dma_start(out=dst, in_=src).then_inc(s_d[key], 16)

    dma(xst[0][:], xs_d[0][:], "x0")
    dma(auxt[:], aux_d[:], "aux")
    dma(cvt2[:, 0, :], cv_d[:], "cv")
    for s in range(1, NS):
        dma(xst[s][:], xs_d[s][:], f"x{s}")

    # ---------------- Act: cv/4 tile, chunked right behind the cv DMA ----------------
    # dummy first so Bacc's auto-inserted act-table load runs at ~700ns,
    # not in front of the first real chunk
    nc.scalar.wait_ge(s_junk, 1)
    nc.scalar.activation(
        out=actscr[:], in_=junk[:, 0, :],
        func=mybir.ActivationFunctionType.Copy, scale=0.25,
    )
    nc.scalar.wait_ge(s_d["cv"], 16)
    for c in range(4):
        nc.scalar.activation(
            out=cvt2[:, 1, 512 * c : 512 * (c + 1)],
            in_=cvt2[:, 0, 512 * c : 512 * (c + 1)],
            func=mybir.ActivationFunctionType.Copy, scale=0.25,
        ).then_inc(s_cv4, 1)

    # ---------------- DVE: masks, one-hots, diag tiles, drains ----------------
    nc.vector.wait_ge(s_iota, 2)
    nc.vector.tensor_copy(out=iota_f[:], in_=iota_i[:])
    nc.vector.tensor_copy(out=ident_f[:], in_=ident_i[:])
    nc.vector.tensor_scalar(
        out=identf[:], in0=ident_f[:], scalar1=0.0, scalar2=None,
        op0=mybir.AluOpType.is_equal,
    )
    nc.vector.tensor_copy(out=ident8, in_=identf[:])
    nc.vector.memset(outsb[:], 0.0)
    # one-hots pair-gated: clsA0 pair k can start as soon as tiles 2k,2k+1
    # are built (s_oh counts pairs; the 5th inc marks the diag tiles)
    nc.vector.wait_ge(s_d["aux"], 16)
    for t in range(NT):
        mm = nc.vector.tensor_scalar(
            out=oh[:, t, :], in0=iota_f[:], scalar1=auxt[:, t : t + 1],
            scalar2=None, op0=mybir.AluOpType.is_equal,
        )
        if t % 2 == 1:
            mm.then_inc(s_oh, 1)
    nc.vector.tensor_scalar(
        out=dg2[:, 0, :], in0=identf[:], scalar1=auxt[:, 8:9], scalar2=None,
        op0=mybir.AluOpType.mult,
    )
    nc.vector.tensor_scalar(
        out=dg2[:, 1, :], in0=identf[:], scalar1=auxt[:, 9:10], scalar2=None,
        op0=mybir.AluOpType.mult,
    ).then_inc(s_oh, 1)

    # drains, all on DVE (the only engine that can read PSUM and do
    # tensor*tensor on real hardware), in stop order; the 128-wide masked
    # gram-diag drain is last. PE stop order: A0,A1,S2..S5 (1..6), clsS6
    # (7th), last gram (8th).
    for s in range(NS):
        nc.vector.wait_ge(s_pe, s + 1)
        nc.vector.scalar_tensor_tensor(
            out=csc[:, 0 : SW[s]], in0=region(s),
            scalar=1.0, in1=cvt2[:, 0, SS[s] : SS[s] + SW[s]],
            op0=mybir.AluOpType.mult, op1=mybir.AluOpType.mult,
            accum_out=outsb[:, 0, 0, s : s + 1],
        )
    nc.vector.wait_ge(s_pe, NS + 1)
    nc.vector.scalar_tensor_tensor(
        out=csc[:, 0:W], in0=ps_g,
        scalar=1.0, in1=ident8,
        op0=mybir.AluOpType.mult, op1=mybir.AluOpType.mult,
        accum_out=outsb[:, 0, 0, NS : NS + 1],
    ).then_inc(s_dve, 1)

    # ---------------- PE: warmup, class+diag chains, gram chain ----------------
    nc.tensor.wait_ge(s_junk, 1)
    for i in range(NWARM):
        nc.tensor.matmul(
            out=ps_j[:], lhsT=junk[:], rhs=junk[:],
            start=(i == 0), stop=(i == NWARM - 1),
            perf_mode=mybir.MatmulPerfMode.DoubleRow,
        )
    nc.tensor.wait_ge(s_oh, 1)

    def diag_mm(s, start, stop=False):
        mm = nc.tensor.matmul(
            out=region(s), lhsT=dg2[:],
            rhs=cvt2[:, :, SS[s] : SS[s] + SW[s]],
            start=start, stop=stop,
            perf_mode=mybir.MatmulPerfMode.DoubleRow,
        )
        if stop:
            mm.then_inc(s_pe, 1)

    def cls_mm(s, k, stop, start=False):
        mm = nc.tensor.matmul(
            out=region(s), lhsT=oh[:, 2 * k : 2 * k + 2, :],
            rhs=xst[s][:, 2 * k : 2 * k + 2, :],
            start=start, stop=stop,
            perf_mode=mybir.MatmulPerfMode.DoubleRow,
        )
        if stop:
            mm.then_inc(s_pe, 1)

    gram_started = False

    def gram_mm(s, b, k, stop=False):
        nonlocal gram_started
        blk = xst[s][:, 2 * k : 2 * k + 2, P * b : P * (b + 1)]
        mm = nc.tensor.matmul(
            out=ps_g, lhsT=blk, rhs=blk,
            start=not gram_started, stop=stop,
            perf_mode=mybir.MatmulPerfMode.DoubleRow,
        )
        gram_started = True
        if stop:
            mm.then_inc(s_pe, 1)

    # slab 0: cls pairs as their one-hots land (chain start on p0), then
    # diag A0 as the chain STOP (gated on its cv/4 chunk)
    nc.tensor.wait_ge(s_d["x0"], 16)
    for k in range(KDR):
        nc.tensor.wait_ge(s_oh, k + 1)
        cls_mm(0, k, stop=False, start=(k == 0))
    nc.tensor.wait_ge(s_oh, 5)
    nc.tensor.wait_ge(s_cv4, 1)
    diag_mm(0, start=False, stop=True)

    # slabs 1..6: cls p0-p2, the region's diag (gated on its cv/4 chunk,
    # mid-chain), then the p3 STOP -- so each region stop lands as early as
    # its last data allows. Gram batches are sequenced into the wait gaps:
    # a slab's grams run AFTER later-slab cls batches whose sems they'd
    # otherwise delay.
    cv4_chunk = {1: 2, 2: 3, 3: 3, 4: 4, 5: 4, 6: 4}
    cv4_seen = 1

    def need_cv4(s):
        nonlocal cv4_seen
        if cv4_chunk[s] > cv4_seen:
            cv4_seen = cv4_chunk[s]
            nc.tensor.wait_ge(s_cv4, cv4_seen)

    def grams(s, stop=False):
        for b in range(GB[s]):
            for k in range(KDR):
                gram_mm(s, b, k, stop=(stop and b == GB[s] - 1 and k == KDR - 1))

    def cls_batch(s):
        nc.tensor.wait_ge(s_d[f"x{s}"], 16)
        for k in range(KDR - 1):
            cls_mm(s, k, stop=False, start=(k == 0))
        need_cv4(s)
        diag_mm(s, start=False)
        cls_mm(s, KDR - 1, stop=True)

    cls_batch(1)
    grams(0)
    cls_batch(2)
    grams(1)
    cls_batch(3)
    grams(2)
    grams(3)
    cls_batch(4)
    grams(4)
    cls_batch(5)
    grams(5)
    cls_batch(6)
    grams(6, stop=True)

    # ---------------- Pool: fire the writeback after the last drains ----------------
    nc.gpsimd.wait_ge(s_prep, 1)
    nc.gpsimd.wait_ge(s_dve, 1)
    nc.gpsimd.trigger_dma(count=1)
    nc.gpsimd.wait_ge(s_dma, 16)

    nc.compile()
    return nc


def _pack_core(x_sh, lab_sh, centers, lo):
    """Per-core input arrays. x_sh/lab_sh already sorted by label."""
    span = int(lab_sh[-1]) - lo + 1
    assert span <= W, f"class window {span} exceeds {W}"
    # xq[p, t, :] = fp8(x_sorted[t*128 + p, :])
    xq = x_sh.reshape(NT, P, D).transpose(1, 0, 2).astype(NP_FP8)
    aux = np.zeros((P, 10), dtype=np.float32)
    aux[:, :NT] = (lab_sh - lo).reshape(NT, P).T
    cnt = np.bincount(lab_sh - lo, minlength=W)[:W]
    assert cnt.max() <= 67, f"class count {cnt.max()} exceeds fp8-exact split"
    aux[:, 8] = (cnt >> 2).astype(np.float32)
    aux[:, 9] = (cnt & 3).astype(np.float32)
    cw = np.zeros((P, D), dtype=np.float64)
    hi = min(lo + W, C)
    cw[: hi - lo] = centers[lo:hi]
    cvq = np.ascontiguousarray((cw * -2.0).astype(np.float32).astype(NP_FP8))
    d = {"aux": aux, "cvq": cvq}
    for s in range(NS):
        d[f"xs{s}"] = np.ascontiguousarray(xq[:, :, SS[s] : SS[s] + SW[s]])
    return d


def make_in_maps(x, labels, centers):
    order = np.argsort(labels, kind="stable")
    xs = x[order]
    ls = labels[order].astype(np.int64)
    in_maps = []
    for c in range(N_CORES):
        sl = slice(c * BS, (c + 1) * BS)
        in_maps.append(_pack_core(xs[sl], ls[sl], centers, int(ls[sl.start])))
    return in_maps


def combine_partials(partials):
    total = 0.0
    for p in partials:
        total += float(np.sum(np.asarray(p, dtype=np.float64)))
    total += float(B) * float(C - 1) * CLIP_LO
    return np.array(total / B, dtype=np.float32)


def kernel(**inputs) -> np.ndarray:
    global _NC
    x = np.ascontiguousarray(np.asarray(inputs["x"], dtype=np.float32))
    labels = np.asarray(inputs["labels"]).astype(np.int64)
    centers = np.ascontiguousarray(np.asarray(inputs["centers"], dtype=np.float32))
    assert x.shape == (B, D) and labels.shape == (B,) and centers.shape == (C, D)

    if _NC is None:
        _NC = build_nc()
    res = run_bass_kernel_spmd(
        _NC, make_in_maps(x, labels, centers), core_ids=list(range(N_CORES))
    )
    return combine_partials([r["part"] for r in res.results])


# revision 7
# speedup vs baseline: 1.0086x; 1.0086x over previous
"""CenterLoss Trainium2 kernel v2 (raw bass, data-parallel over 8 cores).

loss = sum(clip(distmat * onehot(labels), 1e-12, 1e12)) / B with
distmat[i,c] = ||x_i - centers_c||^2. Only (i, labels_i) entries survive the
mask; the B*(C-1) masked zeros contribute exactly 1e-12 each (added on host).

  sum_i d_i = sum_i ||x_i||^2 + sum_c n_c ||c_c||^2 - 2 sum_c <s_c, c_c>

with s = onehot^T @ x, n_c per-core class counts. Host sorts samples by label
so each core's 1024 samples span a <=128-class window; x ships fp8.

v2 vs the Tile baseline (13813ns -> ~11.5us):
* Raw bass, manual semaphores. No TileContext: the Tile epilogue (double
  barrier round after the output DMA, ~720ns) disappears, and the SWDGE
  prepare/trigger path becomes usable.
* Output via kv_writeback prepared early (descriptor gen off the critical
  path) + trigger_dma after the last drain: post-accum tail is
  ~40ns dispatch + 4ns transfer + 917ns DMA-completion sem, vs the HWDGE
  dma_start chain (650 SEQ + 625 gen + 650 DGE delay + copy + 917).
* Column-slice-major streaming: x arrives as 7 slabs (512,512,256,256,256,
  128,128 cols) x all 8 tiles, host-packed contiguous so every DMA moves
  >=1KB/descriptor. Each slab completes a narrow PSUM region whose drain
  (DVE stt) runs while later slabs still stream -- the drain pipeline never
  piles up at the end (the last drain is only 128 wide).
* Center-norm term folded into the class matmuls: a DoubleRow diag pair
  (diag(n>>2), diag(n&3)) x (cv, cv/4) injects (n_c/4)*(-2c) into ps, so
  the single drain ps.cv yields -2<s,c> + n||c||^2. All values fp8e4m3-exact
  (n <= 67; cv/4 is a power-of-two scale computed by the otherwise-idle Act
  engine in 512-col chunks chasing the cv DMA).
* sum||x||^2 on the PE as before: fp8 DoubleRow Gram matmuls of every
  128-col block against itself, all accumulated into one [128,128] PSUM
  whose masked diag drain is the final 128-wide stt.

Per-core output: [128, 8] f32 raw partial columns (one per drain).
Host combine (f64): sum + B*(C-1)*1e-12, divide by B.
"""

import ml_dtypes
import numpy as np

import concourse.bacc as bacc
from concourse import mybir
from concourse.bass_utils import run_bass_kernel_spmd

N_CORES = 8
B = 8192
D = 2048
C = 751
BS = B // N_CORES
P = 128
NT = BS // P          # 8 sample tiles per core
KDR = NT // 2         # 4 DoubleRow pairs
W = 128               # class window width
OUTW = 8
FP8 = mybir.dt.float8e4
F32 = mybir.dt.float32
BF16 = mybir.dt.bfloat16
NP_FP8 = ml_dtypes.float8_e4m3
CLIP_LO = 1e-12

# column slabs: starts/widths; slab s covers x cols [SS[s], SS[s]+SW[s])
SW = [512, 512, 256, 256, 256, 128, 128]
SS = [0, 512, 1024, 1280, 1536, 1792, 1920]
NS = len(SW)
# gram 128-col blocks per slab
GB = [w // P for w in SW]  # [4,4,2,2,2,1,1]
NWARM = 69

_NC = None


def build_nc():
    nc = bacc.Bacc("TRN2", target_bir_lowering=False)

    aux_d = nc.dram_tensor("aux", [P, 10], F32, kind="ExternalInput")
    cv_d = nc.dram_tensor("cvq", [P, D], FP8, kind="ExternalInput")
    xs_d = [
        nc.dram_tensor(f"xs{s}", [P, NT, SW[s]], FP8, kind="ExternalInput")
        for s in range(NS)
    ]
    out_d = nc.dram_tensor("part", [1, P, 1, OUTW], F32, kind="ExternalOutput")

    # SBUF
    auxt = nc.alloc_sbuf_tensor("auxt", [P, 10], F32)
    # cv tile-pair for the diag DoubleRow: tile0 = fp8(-2c) (DMA), tile1 =
    # tile0 * 0.25 written by the otherwise-idle Act engine (exact
    # power-of-two scale). The diag pair (diag(n>>2), diag(n&3)) x
    # (cv, cv/4) injects (n/4)*(-2c) exactly.
    cvt2 = nc.alloc_sbuf_tensor("cvt2", [P, 2, D], FP8)
    actscr = nc.alloc_sbuf_tensor("actscr", [P, W], F32)
    xst = [nc.alloc_sbuf_tensor(f"xst{s}", [P, NT, SW[s]], FP8) for s in range(NS)]
    oh = nc.alloc_sbuf_tensor("oh", [P, NT, W], FP8)
    iota_i = nc.alloc_sbuf_tensor("iota_i", [P, W], mybir.dt.int32)
    iota_f = nc.alloc_sbuf_tensor("iota_f", [P, W], F32)
    # identraw[p, q] = q - p via iota; ident = (identraw == 0). No
    # scalar-pointer operand (a same-engine back-to-back scalar-ptr read
    # races with its producing write on hardware dispatch).
    ident_i = nc.alloc_sbuf_tensor("ident_i", [P, W], mybir.dt.int32)
    ident_f = nc.alloc_sbuf_tensor("ident_f", [P, W], F32)
    identf = nc.alloc_sbuf_tensor("identf", [P, W], F32)
    ident8t = nc.alloc_sbuf_tensor("ident8t", [P, W], FP8)
    ident8 = ident8t[:]
    # diag pair, tile-adjacent for DoubleRow lhsT
    dg2 = nc.alloc_sbuf_tensor("dg2", [P, 2, W], FP8)
    junk = nc.alloc_sbuf_tensor("junk", [P, 2, W], FP8)
    outsb = nc.alloc_sbuf_tensor("outsb", [P, 1, 1, OUTW], F32)
    idxt = nc.alloc_sbuf_tensor("idxt", [P, 1], mybir.dt.int32)
    csc = nc.alloc_sbuf_tensor("csc", [P, 512], BF16)

    # PSUM: one accumulation group per bank. 8 banks: A0, A1 (ps_a),
    # S2..S6 (one each), gram sharing the junk-warmup bank (junk's group
    # closes before the gram group starts).
    ps_a = nc.alloc_psum_tensor("ps_a", [P, 1024], F32)   # A0 [0:512], A1 [512:1024]
    ps_s = [nc.alloc_psum_tensor(f"ps_s{s}", [P, SW[s]], F32) for s in range(2, NS)]
    ps_gj = nc.alloc_psum_tensor("ps_gj", [P, W], F32)

    def region(s):
        """PSUM AP of slab s's class accumulation."""
        if s < 2:
            return ps_a[:, 512 * s : 512 * (s + 1)]
        return ps_s[s - 2][:]

    ps_g = ps_gj[:]
    # junk warmup borrows S5's bank: its group closes ~4.4us before the S5
    # class group opens (widest reopen margin of any bank)
    ps_j = ps_s[3][:]

    # one completion sem PER DMA: the functional executor may complete DMAs
    # out of order, so a single cumulative sem can release a consumer before
    # ITS data has landed (reads uninitialized SBUF -> NaN)
    s_d = {k: nc.alloc_semaphore(f"s_d_{k}")
           for k in ["x0", "aux", "cv"] + [f"x{s}" for s in range(1, NS)]}
    s_junk = nc.alloc_semaphore("s_junk")
    s_iota = nc.alloc_semaphore("s_iota")
    s_oh = nc.alloc_semaphore("s_oh")      # one-hots + diag tiles ready
    s_cv4 = nc.alloc_semaphore("s_cv4")    # cv/4 tile ready
    s_pe = nc.alloc_semaphore("s_pe")      # region stops, cumulative
    s_dve = nc.alloc_semaphore("s_dve")    # DVE's last drain done
    s_prep = nc.alloc_semaphore("s_prep")  # writeback descriptors ready
    s_dma = nc.alloc_semaphore("s_dma")    # writeback done

    # ---------------- Pool: setup + prepped output writeback ----------------
    nc.gpsimd.memset(junk[:], 0.0).then_inc(s_junk, 1)
    nc.gpsimd.iota(iota_i[:], pattern=[[1, W]], base=0, channel_multiplier=0).then_inc(
        s_iota, 1
    )
    nc.gpsimd.iota(ident_i[:], pattern=[[1, W]], base=0, channel_multiplier=-1).then_inc(
        s_iota, 1
    )
    nc.gpsimd.memset(idxt[:], 0)
    nc.gpsimd.kv_writeback(
        out_ap=out_d[:], in_ap=outsb[:], ctx_idxs_ap=idxt[:],
        prepare_only=True, sem=s_dma,
    ).then_inc(s_prep, 1)

    # ---------------- SP: input stream ----------------
    # xs0 first: a small first DMA would leave the wire idle while the
    # second DMA's descriptor-gen (+650ns DGE delay) pipeline fills.
    def dma(dst, src, key):
        nc.sync.dma_start(out=dst, in_=src).then_inc(s_d[key], 16)

    dma(xst[0][:], xs_d[0][:], "x0")
    dma(auxt[:], aux_d[:], "aux")
    dma(cvt2[:, 0, :], cv_d[:], "cv")
    for s in range(1, NS):
        dma(xst[s][:], xs_d[s][:], f"x{s}")

    # ---------------- Act: cv/4 tile, chunked right behind the cv DMA ----------------
    # dummy first so Bacc's auto-inserted act-table load runs at ~700ns,
    # not in front of the first real chunk
    nc.scalar.wait_ge(s_junk, 1)
    nc.scalar.activation(
        out=actscr[:], in_=junk[:, 0, :],
        func=mybir.ActivationFunctionType.Copy, scale=0.25,
    )
    nc.scalar.wait_ge(s_d["cv"], 16)
    for c in range(4):
        nc.scalar.activation(
            out=cvt2[:, 1, 512 * c : 512 * (c + 1)],
            in_=cvt2[:, 0, 512 * c : 512 * (c + 1)],
            func=mybir.ActivationFunctionType.Copy, scale=0.25,
        ).then_inc(s_cv4, 1)

    # ---------------- DVE: masks, one-hots, diag tiles, drains ----------------
    nc.vector.wait_ge(s_iota, 2)
    nc.vector.tensor_copy(out=iota_f[:], in_=iota_i[:])
    nc.vector.tensor_copy(out=ident_f[:], in_=ident_i[:])
    nc.vector.tensor_scalar(
        out=identf[:], in0=ident_f[:], scalar1=0.0, scalar2=None,
        op0=mybir.AluOpType.is_equal,
    )
    nc.vector.tensor_copy(out=ident8, in_=identf[:])
    nc.vector.memset(outsb[:], 0.0)
    # one-hots pair-gated: clsA0 pair k can start as soon as tiles 2k,2k+1
    # are built (s_oh counts pairs; the 5th inc marks the diag tiles)
    nc.vector.wait_ge(s_d["aux"], 16)
    for t in range(NT):
        mm = nc.vector.tensor_scalar(
            out=oh[:, t, :], in0=iota_f[:], scalar1=auxt[:, t : t + 1],
            scalar2=None, op0=mybir.AluOpType.is_equal,
        )
        if t % 2 == 1:
            mm.then_inc(s_oh, 1)
    nc.vector.tensor_scalar(
        out=dg2[:, 0, :], in0=identf[:], scalar1=auxt[:, 8:9], scalar2=None,
        op0=mybir.AluOpType.mult,
    )
    nc.vector.tensor_scalar(
        out=dg2[:, 1, :], in0=identf[:], scalar1=auxt[:, 9:10], scalar2=None,
        op0=mybir.AluOpType.mult,
    ).then_inc(s_oh, 1)

    # drains, all on DVE (the only engine that can read PSUM and do
    # tensor*tensor on real hardware), in stop order; the 128-wide masked
    # gram-diag drain is last. PE stop order: A0,A1,S2..S5 (1..6), clsS6
    # (7th), last gram (8th).
    for s in range(NS):
        nc.vector.wait_ge(s_pe, s + 1)
        nc.vector.scalar_tensor_tensor(
            out=csc[:, 0 : SW[s]], in0=region(s),
            scalar=1.0, in1=cvt2[:, 0, SS[s] : SS[s] + SW[s]],
            op0=mybir.AluOpType.mult, op1=mybir.AluOpType.mult,
            accum_out=outsb[:, 0, 0, s : s + 1],
        )
    nc.vector.wait_ge(s_pe, NS + 1)
    nc.vector.scalar_tensor_tensor(
        out=csc[:, 0:W], in0=ps_g,
        scalar=1.0, in1=ident8,
        op0=mybir.AluOpType.mult, op1=mybir.AluOpType.mult,
        accum_out=outsb[:, 0, 0, NS : NS + 1],
    ).then_inc(s_dve, 1)

    # ---------------- PE: warmup, class+diag chains, gram chain ----------------
    nc.tensor.wait_ge(s_junk, 1)
    for i in range(NWARM):
        nc.tensor.matmul(
            out=ps_j[:], lhsT=junk[:], rhs=junk[:],
            start=(i == 0), stop=(i == NWARM - 1),
            perf_mode=mybir.MatmulPerfMode.DoubleRow,
        )
    nc.tensor.wait_ge(s_oh, 1)

    def diag_mm(s, start, stop=False):
        mm = nc.tensor.matmul(
            out=region(s), lhsT=dg2[:],
            rhs=cvt2[:, :, SS[s] : SS[s] + SW[s]],
            start=start, stop=stop,
            perf_mode=mybir.MatmulPerfMode.DoubleRow,
        )
        if stop:
            mm.then_inc(s_pe, 1)

    def cls_mm(s, k, stop, start=False):
        mm = nc.tensor.matmul(
            out=region(s), lhsT=oh[:, 2 * k : 2 * k + 2, :],
            rhs=xst[s][:, 2 * k : 2 * k + 2, :],
            start=start, stop=stop,
            perf_mode=mybir.MatmulPerfMode.DoubleRow,
        )
        if stop:
            mm.then_inc(s_pe, 1)

    gram_started = False

    def gram_mm(s, b, k, stop=False):
        nonlocal gram_started
        blk = xst[s][:, 2 * k : 2 * k + 2, P * b : P * (b + 1)]
        mm = nc.tensor.matmul(
            out=ps_g, lhsT=blk, rhs=blk,
            start=not gram_started, stop=stop,
            perf_mode=mybir.MatmulPerfMode.DoubleRow,
        )
        gram_started = True
        if stop:
            mm.then_inc(s_pe, 1)

    # slab 0: cls pairs as their one-hots land (chain start on p0), then
    # diag A0 as the chain STOP (gated on its cv/4 chunk)
    nc.tensor.wait_ge(s_d["x0"], 16)
    for k in range(KDR):
        nc.tensor.wait_ge(s_oh, k + 1)
        cls_mm(0, k, stop=False, start=(k == 0))
    nc.tensor.wait_ge(s_oh, 5)
    nc.tensor.wait_ge(s_cv4, 1)
    diag_mm(0, start=False, stop=True)

    # slabs 1..6: cls p0-p2, the region's diag (gated on its cv/4 chunk,
    # mid-chain), then the p3 STOP -- so each region stop lands as early as
    # its last data allows. Gram batches are sequenced into the wait gaps:
    # a slab's grams run AFTER later-slab cls batches whose sems they'd
    # otherwise delay.
    cv4_chunk = {1: 2, 2: 3, 3: 3, 4: 4, 5: 4, 6: 4}
    cv4_seen = 1

    def need_cv4(s):
        nonlocal cv4_seen
        if cv4_chunk[s] > cv4_seen:
            cv4_seen = cv4_chunk[s]
            nc.tensor.wait_ge(s_cv4, cv4_seen)

    def grams(s, stop=False):
        for b in range(GB[s]):
            for k in range(KDR):
                gram_mm(s, b, k, stop=(stop and b == GB[s] - 1 and k == KDR - 1))

    def cls_batch(s):
        nc.tensor.wait_ge(s_d[f"x{s}"], 16)
        for k in range(KDR - 1):
            cls_mm(s, k, stop=False, start=(k == 0))
        need_cv4(s)
        diag_mm(s, start=False)
        cls_mm(s, KDR - 1, stop=True)

    cls_batch(1)
    grams(0)
    cls_batch(2)
    grams(1)
    cls_batch(3)
    grams(2)
    cls_batch(4)
    grams(3)
    cls_batch(5)
    grams(4)
    cls_batch(6)
    grams(5)
    grams(6, stop=True)

    # ---------------- Pool: fire the writeback after the last drains ----------------
    nc.gpsimd.wait_ge(s_prep, 1)
    nc.gpsimd.wait_ge(s_dve, 1)
    nc.gpsimd.trigger_dma(count=1)
    nc.gpsimd.wait_ge(s_dma, 16)

    nc.compile()
    return nc


def _pack_core(x_sh, lab_sh, centers, lo):
    """Per-core input arrays. x_sh/lab_sh already sorted by label."""
    span = int(lab_sh[-1]) - lo + 1
    assert span <= W, f"class window {span} exceeds {W}"
    # xq[p, t, :] = fp8(x_sorted[t*128 + p, :])
    xq = x_sh.reshape(NT, P, D).transpose(1, 0, 2).astype(NP_FP8)
    aux = np.zeros((P, 10), dtype=np.float32)
    aux[:, :NT] = (lab_sh - lo).reshape(NT, P).T
    cnt = np.bincount(lab_sh - lo, minlength=W)[:W]
    assert cnt.max() <= 67, f"class count {cnt.max()} exceeds fp8-exact split"
    aux[:, 8] = (cnt >> 2).astype(np.float32)
    aux[:, 9] = (cnt & 3).astype(np.float32)
    cw = np.zeros((P, D), dtype=np.float64)
    hi = min(lo + W, C)
    cw[: hi - lo] = centers[lo:hi]
    cvq = np.ascontiguousarray((cw * -2.0).astype(np.float32).astype(NP_FP8))
    d = {"aux": aux, "cvq": cvq}
    for s in range(NS):
        d[f"xs{s}"] = np.ascontiguousarray(xq[:, :, SS[s] : SS[s] + SW[s]])
    return d


def make_in_maps(x, labels, centers):
    order = np.argsort(labels, kind="stable")
    xs = x[order]
    ls = labels[order].astype(np.int64)
    in_maps = []
    for c in range(N_CORES):
        sl = slice(c * BS, (c + 1) * BS)
        in_maps.append(_pack_core(xs[sl], ls[sl], centers, int(ls[sl.start])))
    return in_maps


def combine_partials(partials):
    total = 0.0
    for p in partials:
        total += float(np.sum(np.asarray(p, dtype=np.float64)))
    total += float(B) * float(C - 1) * CLIP_LO
    return np.array(total / B, dtype=np.float32)


def kernel(**inputs) -> np.ndarray:
    global _NC
    x = np.ascontiguousarray(np.asarray(inputs["x"], dtype=np.float32))
    labels = np.asarray(inputs["labels"]).astype(np.int64)
    centers = np.ascontiguousarray(np.asarray(inputs["centers"], dtype=np.float32))
    assert x.shape == (B, D) and labels.shape == (B,) and centers.shape == (C, D)

    if _NC is None:
        _NC = build_nc()
    res = run_bass_kernel_spmd(
        _NC, make_in_maps(x, labels, centers), core_ids=list(range(N_CORES))
    )
    return combine_partials([r["part"] for r in res.results])


# revision 8
# speedup vs baseline: 1.0131x; 1.0045x over previous
"""CenterLoss Trainium2 kernel v2 (raw bass, data-parallel over 8 cores).

loss = sum(clip(distmat * onehot(labels), 1e-12, 1e12)) / B with
distmat[i,c] = ||x_i - centers_c||^2. Only (i, labels_i) entries survive the
mask; the B*(C-1) masked zeros contribute exactly 1e-12 each (added on host).

  sum_i d_i = sum_i ||x_i||^2 + sum_c n_c ||c_c||^2 - 2 sum_c <s_c, c_c>

with s = onehot^T @ x, n_c per-core class counts. Host sorts samples by label
so each core's 1024 samples span a <=128-class window; x ships fp8.

v2 vs the Tile baseline (13813ns -> ~11.5us):
* Raw bass, manual semaphores. No TileContext: the Tile epilogue (double
  barrier round after the output DMA, ~720ns) disappears, and the SWDGE
  prepare/trigger path becomes usable.
* Output via kv_writeback prepared early (descriptor gen off the critical
  path) + trigger_dma after the last drain: post-accum tail is
  ~40ns dispatch + 4ns transfer + 917ns DMA-completion sem, vs the HWDGE
  dma_start chain (650 SEQ + 625 gen + 650 DGE delay + copy + 917).
* Column-slice-major streaming: x arrives as 7 slabs (512,512,256,256,256,
  128,128 cols) x all 8 tiles, host-packed contiguous so every DMA moves
  >=1KB/descriptor. Each slab completes a narrow PSUM region whose drain
  (DVE stt) runs while later slabs still stream -- the drain pipeline never
  piles up at the end (the last drain is only 128 wide).
* Center-norm term folded into the class matmuls: a DoubleRow diag pair
  (diag(n>>2), diag(n&3)) x (cv, cv/4) injects (n_c/4)*(-2c) into ps, so
  the single drain ps.cv yields -2<s,c> + n||c||^2. All values fp8e4m3-exact
  (n <= 67; cv/4 is a power-of-two scale computed by the otherwise-idle Act
  engine in 512-col chunks chasing the cv DMA).
* sum||x||^2 on the PE as before: fp8 DoubleRow Gram matmuls of every
  128-col block against itself, all accumulated into one [128,128] PSUM
  whose masked diag drain is the final 128-wide stt.

Per-core output: [128, 8] f32 raw partial columns (one per drain).
Host combine (f64): sum + B*(C-1)*1e-12, divide by B.
"""

import ml_dtypes
import numpy as np

import concourse.bacc as bacc
from concourse import mybir
from concourse.bass_utils import run_bass_kernel_spmd

N_CORES = 8
B = 8192
D = 2048
C = 751
BS = B // N_CORES
P = 128
NT = BS // P          # 8 sample tiles per core
KDR = NT // 2         # 4 DoubleRow pairs
W = 128               # class window width
OUTW = 8
FP8 = mybir.dt.float8e4
F32 = mybir.dt.float32
BF16 = mybir.dt.bfloat16
NP_FP8 = ml_dtypes.float8_e4m3
CLIP_LO = 1e-12

# column slabs: starts/widths; slab s covers x cols [SS[s], SS[s]+SW[s])
SW = [512, 512, 256, 256, 256, 128, 128]
SS = [0, 512, 1024, 1280, 1536, 1792, 1920]
NS = len(SW)
# gram 128-col blocks per slab
GB = [w // P for w in SW]  # [4,4,2,2,2,1,1]
NWARM = 69

_NC = None


def build_nc():
    nc = bacc.Bacc("TRN2", target_bir_lowering=False)

    aux_d = nc.dram_tensor("aux", [P, 10], F32, kind="ExternalInput")
    cv_d = nc.dram_tensor("cvq", [P, D], FP8, kind="ExternalInput")
    xs_d = [
        nc.dram_tensor(f"xs{s}", [P, NT, SW[s]], FP8, kind="ExternalInput")
        for s in range(NS)
    ]
    out_d = nc.dram_tensor("part", [1, P, 1, OUTW], F32, kind="ExternalOutput")

    # SBUF
    auxt = nc.alloc_sbuf_tensor("auxt", [P, 10], F32)
    # cv tile-pair for the diag DoubleRow: tile0 = fp8(-2c) (DMA), tile1 =
    # tile0 * 0.25 written by the otherwise-idle Act engine (exact
    # power-of-two scale). The diag pair (diag(n>>2), diag(n&3)) x
    # (cv, cv/4) injects (n/4)*(-2c) exactly.
    cvt2 = nc.alloc_sbuf_tensor("cvt2", [P, 2, D], FP8)
    actscr = nc.alloc_sbuf_tensor("actscr", [P, W], F32)
    xst = [nc.alloc_sbuf_tensor(f"xst{s}", [P, NT, SW[s]], FP8) for s in range(NS)]
    oh = nc.alloc_sbuf_tensor("oh", [P, NT, W], FP8)
    iota_i = nc.alloc_sbuf_tensor("iota_i", [P, W], mybir.dt.int32)
    iota_f = nc.alloc_sbuf_tensor("iota_f", [P, W], F32)
    # identraw[p, q] = q - p via iota; ident = (identraw == 0). No
    # scalar-pointer operand (a same-engine back-to-back scalar-ptr read
    # races with its producing write on hardware dispatch).
    ident_i = nc.alloc_sbuf_tensor("ident_i", [P, W], mybir.dt.int32)
    ident_f = nc.alloc_sbuf_tensor("ident_f", [P, W], F32)
    identf = nc.alloc_sbuf_tensor("identf", [P, W], F32)
    ident8t = nc.alloc_sbuf_tensor("ident8t", [P, W], FP8)
    ident8 = ident8t[:]
    # diag pair, tile-adjacent for DoubleRow lhsT
    dg2 = nc.alloc_sbuf_tensor("dg2", [P, 2, W], FP8)
    junk = nc.alloc_sbuf_tensor("junk", [P, 2, W], FP8)
    outsb = nc.alloc_sbuf_tensor("outsb", [P, 1, 1, OUTW], F32)
    idxt = nc.alloc_sbuf_tensor("idxt", [P, 1], mybir.dt.int32)
    csc = nc.alloc_sbuf_tensor("csc", [P, 512], BF16)

    # PSUM: one accumulation group per bank. 8 banks: A0, A1 (ps_a),
    # S2..S6 (one each), gram sharing the junk-warmup bank (junk's group
    # closes before the gram group starts).
    ps_a = nc.alloc_psum_tensor("ps_a", [P, 1024], F32)   # A0 [0:512], A1 [512:1024]
    ps_s = [nc.alloc_psum_tensor(f"ps_s{s}", [P, SW[s]], F32) for s in range(2, NS)]
    ps_gj = nc.alloc_psum_tensor("ps_gj", [P, W], F32)

    def region(s):
        """PSUM AP of slab s's class accumulation."""
        if s < 2:
            return ps_a[:, 512 * s : 512 * (s + 1)]
        return ps_s[s - 2][:]

    ps_g = ps_gj[:]
    # junk warmup borrows S5's bank: its group closes ~4.4us before the S5
    # class group opens (widest reopen margin of any bank)
    ps_j = ps_s[3][:]

    # one completion sem PER DMA: the functional executor may complete DMAs
    # out of order, so a single cumulative sem can release a consumer before
    # ITS data has landed (reads uninitialized SBUF -> NaN)
    s_d = {k: nc.alloc_semaphore(f"s_d_{k}")
           for k in ["x0", "aux", "cv"] + [f"x{s}" for s in range(1, NS)]}
    s_junk = nc.alloc_semaphore("s_junk")
    s_iota = nc.alloc_semaphore("s_iota")
    s_oh = nc.alloc_semaphore("s_oh")      # one-hots + diag tiles ready
    s_cv4 = nc.alloc_semaphore("s_cv4")    # cv/4 tile ready
    s_pe = nc.alloc_semaphore("s_pe")      # region stops, cumulative
    s_dve = nc.alloc_semaphore("s_dve")    # DVE's last drain done
    s_prep = nc.alloc_semaphore("s_prep")  # writeback descriptors ready
    s_dma = nc.alloc_semaphore("s_dma")    # writeback done

    # ---------------- Pool: setup + prepped output writeback ----------------
    nc.gpsimd.memset(junk[:], 0.0).then_inc(s_junk, 1)
    nc.gpsimd.iota(iota_i[:], pattern=[[1, W]], base=0, channel_multiplier=0).then_inc(
        s_iota, 1
    )
    nc.gpsimd.iota(ident_i[:], pattern=[[1, W]], base=0, channel_multiplier=-1).then_inc(
        s_iota, 1
    )
    nc.gpsimd.memset(idxt[:], 0)
    nc.gpsimd.kv_writeback(
        out_ap=out_d[:], in_ap=outsb[:], ctx_idxs_ap=idxt[:],
        prepare_only=True, sem=s_dma,
    ).then_inc(s_prep, 1)

    # ---------------- SP: input stream ----------------
    # xs0 first: a small first DMA would leave the wire idle while the
    # second DMA's descriptor-gen (+650ns DGE delay) pipeline fills.
    def dma(dst, src, key):
        nc.sync.dma_start(out=dst, in_=src).then_inc(s_d[key], 16)

    dma(xst[0][:], xs_d[0][:], "x0")
    dma(auxt[:], aux_d[:], "aux")
    dma(cvt2[:, 0, :], cv_d[:], "cv")
    for s in range(1, NS):
        dma(xst[s][:], xs_d[s][:], f"x{s}")

    # ---------------- Act: cv/4 tile, chunked right behind the cv DMA ----------------
    # dummy first so Bacc's auto-inserted act-table load runs at ~700ns,
    # not in front of the first real chunk
    nc.scalar.wait_ge(s_junk, 1)
    nc.scalar.activation(
        out=actscr[:], in_=junk[:, 0, :],
        func=mybir.ActivationFunctionType.Copy, scale=0.25,
    )
    nc.scalar.wait_ge(s_d["cv"], 16)
    for c in range(4):
        nc.scalar.activation(
            out=cvt2[:, 1, 512 * c : 512 * (c + 1)],
            in_=cvt2[:, 0, 512 * c : 512 * (c + 1)],
            func=mybir.ActivationFunctionType.Copy, scale=0.25,
        ).then_inc(s_cv4, 1)

    # ---------------- DVE: masks, one-hots, diag tiles, drains ----------------
    nc.vector.wait_ge(s_iota, 2)
    nc.vector.tensor_copy(out=iota_f[:], in_=iota_i[:])
    nc.vector.tensor_copy(out=ident_f[:], in_=ident_i[:])
    nc.vector.tensor_scalar(
        out=identf[:], in0=ident_f[:], scalar1=0.0, scalar2=None,
        op0=mybir.AluOpType.is_equal,
    )
    nc.vector.tensor_copy(out=ident8, in_=identf[:])
    nc.vector.memset(outsb[:], 0.0)
    # one-hots pair-gated: clsA0 pair k can start as soon as tiles 2k,2k+1
    # are built (s_oh counts pairs; the 5th inc marks the diag tiles)
    nc.vector.wait_ge(s_d["aux"], 16)
    for t in range(NT):
        mm = nc.vector.tensor_scalar(
            out=oh[:, t, :], in0=iota_f[:], scalar1=auxt[:, t : t + 1],
            scalar2=None, op0=mybir.AluOpType.is_equal,
        )
        if t % 2 == 1:
            mm.then_inc(s_oh, 1)
    nc.vector.tensor_scalar(
        out=dg2[:, 0, :], in0=identf[:], scalar1=auxt[:, 8:9], scalar2=None,
        op0=mybir.AluOpType.mult,
    )
    nc.vector.tensor_scalar(
        out=dg2[:, 1, :], in0=identf[:], scalar1=auxt[:, 9:10], scalar2=None,
        op0=mybir.AluOpType.mult,
    ).then_inc(s_oh, 1)

    # drains, all on DVE (the only engine that can read PSUM and do
    # tensor*tensor on real hardware), in stop order; the 128-wide masked
    # gram-diag drain is last. PE stop order: A0,A1,S2..S5 (1..6), clsS6
    # (7th), last gram (8th).
    for s in range(NS):
        nc.vector.wait_ge(s_pe, s + 1)
        nc.vector.scalar_tensor_tensor(
            out=csc[:, 0 : SW[s]], in0=region(s),
            scalar=1.0, in1=cvt2[:, 0, SS[s] : SS[s] + SW[s]],
            op0=mybir.AluOpType.mult, op1=mybir.AluOpType.mult,
            accum_out=outsb[:, 0, 0, s : s + 1],
        )
    nc.vector.wait_ge(s_pe, NS + 1)
    nc.vector.scalar_tensor_tensor(
        out=csc[:, 0:W], in0=ps_g,
        scalar=1.0, in1=ident8,
        op0=mybir.AluOpType.mult, op1=mybir.AluOpType.mult,
        accum_out=outsb[:, 0, 0, NS : NS + 1],
    ).then_inc(s_dve, 1)

    # ---------------- PE: warmup, class+diag chains, gram chain ----------------
    nc.tensor.wait_ge(s_junk, 1)
    for i in range(NWARM):
        nc.tensor.matmul(
            out=ps_j[:], lhsT=junk[:], rhs=junk[:],
            start=(i == 0), stop=(i == NWARM - 1),
            perf_mode=mybir.MatmulPerfMode.DoubleRow,
        )
    nc.tensor.wait_ge(s_oh, 1)

    def diag_mm(s, start, stop=False):
        mm = nc.tensor.matmul(
            out=region(s), lhsT=dg2[:],
            rhs=cvt2[:, :, SS[s] : SS[s] + SW[s]],
            start=start, stop=stop,
            perf_mode=mybir.MatmulPerfMode.DoubleRow,
        )
        if stop:
            mm.then_inc(s_pe, 1)

    def cls_mm(s, k, stop, start=False):
        mm = nc.tensor.matmul(
            out=region(s), lhsT=oh[:, 2 * k : 2 * k + 2, :],
            rhs=xst[s][:, 2 * k : 2 * k + 2, :],
            start=start, stop=stop,
            perf_mode=mybir.MatmulPerfMode.DoubleRow,
        )
        if stop:
            mm.then_inc(s_pe, 1)

    gram_started = False

    def gram_mm(s, b, k, stop=False):
        nonlocal gram_started
        blk = xst[s][:, 2 * k : 2 * k + 2, P * b : P * (b + 1)]
        mm = nc.tensor.matmul(
            out=ps_g, lhsT=blk, rhs=blk,
            start=not gram_started, stop=stop,
            perf_mode=mybir.MatmulPerfMode.DoubleRow,
        )
        gram_started = True
        if stop:
            mm.then_inc(s_pe, 1)

    # slab 0: cls pairs as their one-hots land (chain start on p0), then
    # diag A0 as the chain STOP (gated on its cv/4 chunk)
    nc.tensor.wait_ge(s_d["x0"], 16)
    for k in range(KDR):
        nc.tensor.wait_ge(s_oh, k + 1)
        cls_mm(0, k, stop=False, start=(k == 0))
    nc.tensor.wait_ge(s_oh, 5)
    nc.tensor.wait_ge(s_cv4, 1)
    diag_mm(0, start=False, stop=True)

    # slabs 1..6: cls p0-p2, the region's diag (gated on its cv/4 chunk,
    # mid-chain), then the p3 STOP -- so each region stop lands as early as
    # its last data allows. Gram batches are sequenced into the wait gaps:
    # a slab's grams run AFTER later-slab cls batches whose sems they'd
    # otherwise delay.
    cv4_chunk = {1: 2, 2: 3, 3: 3, 4: 4, 5: 4, 6: 4}
    cv4_seen = 1

    def need_cv4(s):
        nonlocal cv4_seen
        if cv4_chunk[s] > cv4_seen:
            cv4_seen = cv4_chunk[s]
            nc.tensor.wait_ge(s_cv4, cv4_seen)

    def grams(s, stop=False):
        for b in range(GB[s]):
            for k in range(KDR):
                gram_mm(s, b, k, stop=(stop and b == GB[s] - 1 and k == KDR - 1))

    def cls_batch(s):
        # diag first (chain start): it only needs its cv/4 chunk, which is
        # ready long before the slab's x data -- keeps the region STOP at
        # x-sem + cls work only
        need_cv4(s)
        diag_mm(s, start=True)
        nc.tensor.wait_ge(s_d[f"x{s}"], 16)
        for k in range(KDR - 1):
            cls_mm(s, k, stop=False)
        cls_mm(s, KDR - 1, stop=True)

    cls_batch(1)
    grams(0)
    cls_batch(2)
    grams(1)
    cls_batch(3)
    grams(2)
    cls_batch(4)
    grams(3)
    cls_batch(5)
    grams(4)
    cls_batch(6)
    grams(5)
    grams(6, stop=True)

    # ---------------- Pool: fire the writeback after the last drains ----------------
    nc.gpsimd.wait_ge(s_prep, 1)
    nc.gpsimd.wait_ge(s_dve, 1)
    nc.gpsimd.trigger_dma(count=1)
    nc.gpsimd.wait_ge(s_dma, 16)

    nc.compile()
    return nc


def _pack_core(x_sh, lab_sh, centers, lo):
    """Per-core input arrays. x_sh/lab_sh already sorted by label."""
    span = int(lab_sh[-1]) - lo + 1
    assert span <= W, f"class window {span} exceeds {W}"
    # xq[p, t, :] = fp8(x_sorted[t*128 + p, :])
    xq = x_sh.reshape(NT, P, D).transpose(1, 0, 2).astype(NP_FP8)
    aux = np.zeros((P, 10), dtype=np.float32)
    aux[:, :NT] = (lab_sh - lo).reshape(NT, P).T
    cnt = np.bincount(lab_sh - lo, minlength=W)[:W]
    assert cnt.max() <= 67, f"class count {cnt.max()} exceeds fp8-exact split"
    aux[:, 8] = (cnt >> 2).astype(np.float32)
    aux[:, 9] = (cnt & 3).astype(np.float32)
    cw = np.zeros((P, D), dtype=np.float64)
    hi = min(lo + W, C)
    cw[: hi - lo] = centers[lo:hi]
    cvq = np.ascontiguousarray((cw * -2.0).astype(np.float32).astype(NP_FP8))
    d = {"aux": aux, "cvq": cvq}
    for s in range(NS):
        d[f"xs{s}"] = np.ascontiguousarray(xq[:, :, SS[s] : SS[s] + SW[s]])
    return d


def make_in_maps(x, labels, centers):
    order = np.argsort(labels, kind="stable")
    xs = x[order]
    ls = labels[order].astype(np.int64)
    in_maps = []
    for c in range(N_CORES):
        sl = slice(c * BS, (c + 1) * BS)
        in_maps.append(_pack_core(xs[sl], ls[sl], centers, int(ls[sl.start])))
    return in_maps


def combine_partials(partials):
    total = 0.0
    for p in partials:
        total += float(np.sum(np.asarray(p, dtype=np.float64)))
    total += float(B) * float(C - 1) * CLIP_LO
    return np.array(total / B, dtype=np.float32)


def kernel(**inputs) -> np.ndarray:
    global _NC
    x = np.ascontiguousarray(np.asarray(inputs["x"], dtype=np.float32))
    labels = np.asarray(inputs["labels"]).astype(np.int64)
    centers = np.ascontiguousarray(np.asarray(inputs["centers"], dtype=np.float32))
    assert x.shape == (B, D) and labels.shape == (B,) and centers.shape == (C, D)

    if _NC is None:
        _NC = build_nc()
    res = run_bass_kernel_spmd(
        _NC, make_in_maps(x, labels, centers), core_ids=list(range(N_CORES))
    )
    return combine_partials([r["part"] for r in res.results])
